# revision 5
# baseline (speedup 1.0000x reference)
"""Elementwise hard-clip kernel for Trainium2 (8 NeuronCores, SPMD).

Computes y = clip(x, -0.5, 0.5) for x of shape (32, 2, 1048576) float32.

Strategy: the 2e-2 rel-err budget admits bf16 (~4e-3), so the host casts
x to bf16 (RNE) and the device streams bf16: flatten to 67,108,864
elements, shard contiguously across 8 cores (8,388,608 elements = 16 MiB
bf16 per core).  The whole per-core shard fits in SBUF ([128 x 65536]
bf16 = 128 KiB/partition), so there is no slot ring and no WAR hazard:
all loads are issued back-to-back up front (split across both HWDGE
rings to cut issue latency), VectorE clips each tile as it lands (fused
min/max tensor_scalar), and stores chase the clips on the ACT ring.  The
host widens the bf16 result back to f32.

Why bf16 in HBM: the ~435 GB/s per-core ceiling (16 SDMA engines x 32 B
x 850 MHz) binds total bytes through the DMA engines, on both the SBUF
AXI and effective HBM side (measured: an f32 pipeline saturates at ~430
combined; SWDGE cast-in-DMA does not beat it because the f32 side of a
casting transfer still counts).  Halving bytes in HBM halves the
roofline: 32 MiB/core total -> ~74 us vs ~154 us for f32.  A shorter
kernel also mostly dodges the HBM-stack-mate contention window that made
the f32 version bimodal (165/190 us).

Raw bass (no TileContext): hand-rolled semaphore pipeline avoids Tile's
~8 us EVSEM exit barrier; enable_partition_id=False drops the per-engine
partition-id TENSOR_LOADs from the preamble.
"""

from contextlib import ExitStack

import ml_dtypes
import numpy as np

import concourse.bass as bass
import concourse.mybir as mybir
from concourse.bass_utils import run_bass_kernel_spmd

N_CORES = 8
FULL_SHAPE = (32, 2, 1048576)
TOTAL = FULL_SHAPE[0] * FULL_SHAPE[1] * FULL_SHAPE[2]  # 67,108,864
PER_CORE = TOTAL // N_CORES  # 8,388,608
P = 128
TOTF = PER_CORE // P  # 65536 bf16 elems per partition = 128 KiB
# Per-tile elements per partition.  Small first tile so the first
# clip+store enter the pipe early; small last tile so the final
# load->clip->store chain drains quickly.  Keep per-partition runs
# >= 8 KiB (4096 bf16): smaller runs fall off the 16-engine descriptor
# spray and serialize onto one SDMA engine.
FREES = [4096, 16384, 16384, 16384, 8192, 4096]
NTILES = len(FREES)
OFFS = [sum(FREES[:i]) for i in range(NTILES)]
assert sum(FREES) == TOTF

LO = -0.5
HI = 0.5

_nc_cache = None


def _build():
    nc = bass.Bass(target_bir_lowering=False, enable_partition_id=False)
    x = nc.dram_tensor("x", [PER_CORE], mybir.dt.bfloat16, kind="ExternalInput")
    y = nc.dram_tensor("y", [PER_CORE], mybir.dt.bfloat16, kind="ExternalOutput")

    # Contiguous per-tile DRAM blocks, partition-major inside each block.
    def dram_tile(t, i):
        return bass.AP(t, OFFS[i] * P, [[FREES[i], P], [1, FREES[i]]])

    with (
        nc.Block(no_gpsimd_drain=True) as block,
        ExitStack() as es,
    ):
        ld_s = [es.enter_context(nc.semaphore(f"ld{i}")) for i in range(NTILES)]
        st = es.enter_context(nc.semaphore("st"))
        cp = es.enter_context(nc.semaphore("cp"))
        buf = es.enter_context(nc.sbuf_tensor("buf", [P, TOTF], mybir.dt.bfloat16))

        def tile(i):
            return buf[:, OFFS[i] : OFFS[i] + FREES[i]]

        # Whole shard is SBUF-resident: no WAR waits anywhere.  Split the
        # load issue across both HWDGE rings so descriptor generation
        # (~0.6 us per DMA instruction) doesn't serialize the ramp.
        @block.sync
        def _(sync):
            for i in range(0, NTILES, 2):
                sync.dma_start(tile(i), dram_tile(x, i)).then_inc(ld_s[i], 16)

        @block.vector
        def _(vector):
            for i in range(NTILES):
                vector.wait_ge(ld_s[i], 16)
                s = tile(i)
                vector.tensor_scalar(
                    s, s, HI, LO, mybir.AluOpType.min, mybir.AluOpType.max
                )
                # drain-then-inc: fence the DVE datapath so the store DMA
                # (AXI side) sees the writes before cp releases it
                vector.drain(fusable=False).then_inc(cp, 1)

        @block.scalar
        def _(scalar):
            for i in range(1, NTILES, 2):
                scalar.dma_start(tile(i), dram_tile(x, i)).then_inc(ld_s[i], 16)
            for i in range(NTILES):
                # cp is incremented in DVE stream order -> cumulative is safe
                scalar.wait_ge(cp, i + 1)
                scalar.dma_start(dram_tile(y, i), tile(i)).then_inc(st, 16)
            # total count: 16*NTILES increments <=> every store landed
            scalar.wait_ge(st, 16 * NTILES)

    nc.finalize()
    return nc


def _shards(x):
    xb = np.asarray(x).astype(ml_dtypes.bfloat16)
    return np.ascontiguousarray(xb).reshape(N_CORES, PER_CORE)


def kernel(x):
    global _nc_cache
    shards = _shards(x)
    if _nc_cache is None:
        _nc_cache = _build()
    res = run_bass_kernel_spmd(
        _nc_cache,
        [{"x": shards[i]} for i in range(N_CORES)],
        core_ids=list(range(N_CORES)),
    )
    out = np.concatenate([r["y"] for r in res.results])
    return out.astype(np.float32).reshape(FULL_SHAPE)
